# revision 1
# baseline (speedup 1.0000x reference)
"""GAT layer kernel for Trainium2 (8 NeuronCores, SPMD data-parallel over B).

Reference computation (per (b,t) slice, N=512 nodes, D=F=128):
    h = x_bt @ W                                  [N, F]
    e_src = h @ (W a_src) ... computed as x_bt @ ws, ws = W @ a_src (host)
    e_dst = x_bt @ wd, wd = W @ a_dst
    e[i,j] = leaky_relu(e_src[i] + e_dst[j], 0.2)
    e masked by adj|I, row-softmax, out = elu(alpha @ h)

Device dataflow per (b,t), mostly in eT = e^T [j, i] orientation so the
aggregation can run as PSUM-accumulated matmuls with j as contraction dim:
    xT   : PE transposes of x chunks        [d, n]
    ev   : [e_src; e_dst] = WSD.T @ xT      [2, n] (one f32r matmul)
    h    : W-proj from xT slices            [j, f] chunks
    eadd : PSUM prefilled with mask bias (0/-1e9) via identity matmul,
           then K=2 rank-2 matmul accumulates ed[j]+es[i]
    Prelu(0.2) -> Exp (in place)   => E = masked exp(e)^T  [j, i]
    s    : ones.T @ E  (row sums of e)      [1, i]
    uT   : h.T @ E accumulated              [f, i]
    U    : PE transposes back               [i, f] chunks; out = elu(U / s)

Sharding: B=16 batches over 8 cores (2 per core), T=12 inside.
"""

import numpy as np

B, N, T, D, F = 16, 512, 12, 128, 128
NCORES = 8
B_PER_CORE = B // NCORES
NCH = N // 128  # 4 chunks of 128 nodes


def _build_program(reps=1):
    import concourse.bacc as bacc
    import concourse.tile as tile
    from concourse import mybir

    import os
    F32 = mybir.dt.float32
    F32R = mybir.dt.float32r
    AF = mybir.ActivationFunctionType
    ALU = mybir.AluOpType

    nc = bacc.Bacc()
    LEAKY_FUNC = (mybir.ActivationFunctionType.Relu
                  if os.environ.get("GAT_SIM_RELU") == "1"
                  else mybir.ActivationFunctionType.Prelu)

    x_h = nc.declare_dram_parameter("x", [B_PER_CORE, N, T, D], F32R, isOutput=False)
    w_h = nc.declare_dram_parameter("w", [D, F], F32R, isOutput=False)
    wsd_h = nc.declare_dram_parameter("wsd", [D, 2], F32R, isOutput=False)
    maskt_h = nc.declare_dram_parameter("maskt", [NCH, 128, N], F32R, isOutput=False)
    identr_h = nc.declare_dram_parameter("identr", [128, 128], F32R, isOutput=False)
    ident_h = nc.declare_dram_parameter("ident", [128, 128], F32, isOutput=False)
    ones_h = nc.declare_dram_parameter("onescol", [128, 1], F32R, isOutput=False)
    sel_h = nc.declare_dram_parameter("sel", [2, 2], F32, isOutput=False)
    out_h = nc.declare_dram_parameter("out", [B_PER_CORE, N, T, F], F32, isOutput=True)

    NBT = B_PER_CORE * T

    with tile.TileContext(nc) as tc:
        with (
            tc.tile_pool(name="consts", bufs=1) as consts,
            tc.tile_pool(name="xbuf", bufs=1) as xbuf,
            tc.tile_pool(name="work", bufs=int(os.environ.get("K_WORK", "5"))) as work,
            tc.tile_pool(name="big", bufs=int(os.environ.get("K_BIG", "5"))) as big,
            tc.tile_pool(name="mmA_ps", bufs=int(os.environ.get("K_MMA", "1")), space="PSUM") as mmA_ps,
            tc.tile_pool(name="mmB_ps", bufs=int(os.environ.get("K_MMB", "4")), space="PSUM") as mmB_ps,
            tc.tile_pool(name="eadd_ps", bufs=int(os.environ.get("K_EADD", "1")), space="PSUM") as eadd_ps,
            tc.tile_pool(name="tiny_ps", bufs=int(os.environ.get("K_TINY", "1")), space="PSUM") as tiny_ps,
        ):
            w_sb = consts.tile([D, F], F32R)
            wsd_sb = consts.tile([D, 2], F32R)
            mask_sb = consts.tile([128, NCH, N], F32R)
            id_sb = consts.tile([128, 128], F32)
            idr_sb = consts.tile([128, 128], F32R)
            ones_sb = consts.tile([128, 1], F32R)
            sel_sb = consts.tile([2, 2], F32)
            nc.sync.dma_start(out=w_sb, in_=w_h[:, :])
            nc.sync.dma_start(out=wsd_sb, in_=wsd_h[:, :])
            for c in range(NCH):
                nc.sync.dma_start(out=mask_sb[:, c, :], in_=maskt_h[c, :, :])
            nc.sync.dma_start(out=id_sb, in_=ident_h[:, :])
            nc.sync.dma_start(out=idr_sb, in_=identr_h[:, :])
            nc.sync.dma_start(out=ones_sb, in_=ones_h[:, :])
            nc.sync.dma_start(out=sel_sb, in_=sel_h[:, :])

            # ---- preload ALL of x, split finely so the first (b,t) can
            #      start after ~1.5MB instead of 6.3MB ----
            x_all = []
            TH = T // 2
            for b in range(B_PER_CORE):
                xt = xbuf.tile([128, NCH, T, D], F32R, tag=f"x{b}")
                x_all.append(xt)
            for b in range(B_PER_CORE):
                for th in range(2):
                    for c in range(NCH):
                        nc.sync.dma_start(
                            out=x_all[b][:, c, th * TH:(th + 1) * TH, :],
                            in_=x_h[b, c * 128:(c + 1) * 128,
                                    th * TH:(th + 1) * TH, :])

            # per-bt state carried between pipeline stages
            st = [dict() for _ in range(NBT)]

            def stage1(k):
                b, t = divmod(k, T)
                # transpose x -> xT [d, n]
                xT_ps = mmA_ps.tile([128, NCH, 128], F32R, tag="mmA")
                for c in range(NCH):
                    nc.tensor.transpose(xT_ps[:, c, :], x_all[b][:, c, t, :], idr_sb)
                xT_sb = work.tile([128, NCH, 128], F32R, tag="xT_sb")
                nc.vector.tensor_copy(xT_sb, xT_ps)

                # ev rows [es; ed] -> ev_rhs = [es; ones], ev_lhs = [ones; ed]
                ev_ps = tiny_ps.tile([2, N], F32, tag="tiny")
                nc.tensor.matmul(
                    ev_ps, wsd_sb, xT_sb.rearrange("p a b -> p (a b)"),
                    start=True, stop=True)
                ev_rhs = work.tile([2, N], F32R, tag="ev_rhs")
                ev_lhs = work.tile([2, N], F32R, tag="ev_lhs")
                nc.vector.tensor_scalar(
                    ev_rhs, ev_ps, sel_sb[:, 0:1], sel_sb[:, 1:2],
                    ALU.mult, ALU.add)
                nc.vector.tensor_scalar(
                    ev_lhs, ev_ps, sel_sb[:, 1:2], sel_sb[:, 0:1],
                    ALU.mult, ALU.add)

                # h chunks [j, f]
                h_ps = mmA_ps.tile([128, NCH, F], F32, tag="mmA")
                for c in range(NCH):
                    nc.tensor.matmul(h_ps[:, c, :], xT_sb[:, c, :], w_sb,
                                     start=True, stop=True)
                h_sb = work.tile([128, NCH, F], F32R, tag="h_sb")
                nc.vector.tensor_copy(h_sb, h_ps)

                # eadd halves -> Prelu -> z_sb; then exp+mask in place
                z_sb = big.tile([128, NCH, N], F32R, tag="z_sb")
                for ha in range(2):
                    z_ps = eadd_ps.tile([128, 2, N], F32, tag="eadd")
                    for ci in range(2):
                        c = 2 * ha + ci
                        nc.tensor.matmul(
                            z_ps[:, ci, :], idr_sb, mask_sb[:, c, :],
                            start=True, stop=False)
                        nc.tensor.matmul(
                            z_ps[:, ci, :], ev_lhs[:, c * 128:(c + 1) * 128],
                            ev_rhs, start=False, stop=True)
                    nc.scalar.activation(z_sb[:, 2 * ha:2 * ha + 2, :], z_ps,
                                         LEAKY_FUNC, alpha=0.2)
                nc.scalar.activation(z_sb, z_sb, AF.Exp)
                st[k]["h_sb"] = h_sb
                st[k]["z_sb"] = z_sb

            def stage2(k):
                h_sb, z_sb = st[k]["h_sb"], st[k]["z_sb"]
                s_ps = tiny_ps.tile([1, N], F32, tag="tiny")
                uT_ps = mmB_ps.tile([128, N], F32, tag="mmB")
                for c in range(NCH):
                    nc.tensor.matmul(s_ps, ones_sb, z_sb[:, c, :],
                                     start=(c == 0), stop=(c == NCH - 1))
                for c in range(NCH):
                    nc.tensor.matmul(uT_ps, h_sb[:, c, :], z_sb[:, c, :],
                                     start=(c == 0), stop=(c == NCH - 1))

                s_row = work.tile([1, N], F32, tag="s_row")
                nc.vector.tensor_copy(s_row, s_ps)
                sc_ps = tiny_ps.tile([128, NCH], F32, tag="tiny")
                for c in range(NCH):
                    nc.tensor.transpose(
                        sc_ps[:, c:c + 1], s_row[0:1, c * 128:(c + 1) * 128],
                        id_sb[0:1, 0:1])
                sc_sb = work.tile([128, NCH], F32, tag="sc_sb")
                nc.vector.tensor_copy(sc_sb, sc_ps)
                r_cols = work.tile([128, NCH], F32, tag="r_cols")
                nc.vector.reciprocal_approx_fast(r_cols, sc_sb)

                uT_sb = work.tile([128, N], F32R, tag="uT_sb")
                nc.vector.tensor_copy(uT_sb, uT_ps)
                u_ps = mmB_ps.tile([128, NCH, F], F32R, tag="mmB")
                for c in range(NCH):
                    nc.tensor.transpose(
                        u_ps[:, c, :], uT_sb[:, c * 128:(c + 1) * 128], idr_sb)
                st[k]["u_ps"] = u_ps
                st[k]["r_cols"] = r_cols

            def stage3(k):
                b, t = divmod(k, T)
                u_ps, r_cols = st[k]["u_ps"], st[k]["r_cols"]
                t_sb = work.tile([128, NCH, F], F32, tag="t_sb")
                m_sb = work.tile([128, NCH, F], F32, tag="m_sb")
                for c in range(NCH):
                    nc.scalar.activation(
                        t_sb[:, c, :], u_ps[:, c, :], AF.Exp,
                        scale=r_cols[:, c:c + 1])
                    nc.vector.tensor_scalar(
                        m_sb[:, c, :], u_ps[:, c, :],
                        r_cols[:, c:c + 1], 0.0, ALU.mult, ALU.max)
                e1_sb = work.tile([128, NCH, F], F32, tag="e1_sb")
                nc.vector.tensor_scalar(
                    e1_sb, t_sb, 1.0, -1.0, ALU.min, ALU.add)
                o_sb = work.tile([128, NCH, F], F32, tag="o_sb")
                nc.gpsimd.tensor_tensor(
                    out=o_sb, in0=m_sb, in1=e1_sb, op=ALU.add)
                o_dst = out_h[b, :, t, :].rearrange("(c p) f -> p c f", p=128)
                nc.sync.dma_start(out=o_dst, in_=o_sb)
                st[k].clear()

            # software-pipelined emission with stage lag
            LAG = int(os.environ.get("K_LAG", "3"))

            def body(_iv=None, unroll=1):
                for k in range(NBT + 2 * LAG):
                    if k < NBT:
                        stage1(k)
                    if LAG <= k < NBT + LAG:
                        stage2(k - LAG)
                    if k >= 2 * LAG:
                        stage3(k - 2 * LAG)

            if reps == 1:
                body()
            else:
                with tc.For_i(0, reps, 1) as _iv:
                    body(_iv)

    nc.finalize()
    return nc


def kernel(x, W, a_src, a_dst, adj):
    from concourse.bass_utils import run_bass_kernel_spmd

    x = np.ascontiguousarray(x, dtype=np.float32)
    W = np.ascontiguousarray(W, dtype=np.float32)
    a_src = np.asarray(a_src, dtype=np.float32)
    a_dst = np.asarray(a_dst, dtype=np.float32)
    adj = np.asarray(adj)

    mask = np.where((adj > 0) | np.eye(N, dtype=bool), 0.0, -1e9).astype(np.float32)  # [i, j]
    maskt = np.ascontiguousarray(mask.T.reshape(NCH, 128, N))      # [jc, jl, i]
    wsd = np.ascontiguousarray(np.stack([W @ a_src, W @ a_dst], axis=1))
    ident = np.eye(128, dtype=np.float32)
    onescol = np.ones((128, 1), dtype=np.float32)
    sel = np.array([[1.0, 0.0], [0.0, 1.0]], dtype=np.float32)

    nc = _build_program()

    in_maps = []
    for c in range(NCORES):
        in_maps.append({
            "x": np.ascontiguousarray(x[c * B_PER_CORE:(c + 1) * B_PER_CORE]),
            "w": W, "wsd": wsd, "maskt": maskt, "ident": ident,
            "identr": ident, "onescol": onescol, "sel": sel,
        })

    res = run_bass_kernel_spmd(nc, in_maps, list(range(NCORES)))
    out = np.concatenate([res.results[c]["out"] for c in range(NCORES)], axis=0)
    return out  # [B, N, T, F]

